# revision 3
# baseline (speedup 1.0000x reference)
"""nn_AttentionHeads_52269751992484 — Trainium2 Bass kernel (8 NeuronCores).

Multi-head attention (non-causal, 16 heads of 64), B=8, T=1024, C=1024.
Sharding: data-parallel over batch — one batch element per NeuronCore, no
collectives. Inside each core (all matmuls fp16 with fp32 PSUM accumulate):

  Q^T = Wq^T x^T (+bq)   K^T likewise       [C, T], head-pair partition tiles
  V   = x Wv (+bv) with a 64-wide all-ones block appended ([128, C+64] tiles)
  per head pair: S^T[tk] = K_h^T-chunk^T @ Q_h^T  (two heads row-packed in the
    PE array via tile_position), exp(S/8) on the scalar engine PSUM->SBUF fp16
  per head: one accumulation group [V_h | ones]^T @ expS gives O' on psum
    partitions 0:64 and the softmax denominators broadcast on 64:128;
    reciprocal_approx_fast + tensor_mul normalize; DMA out per-head O^T.

Host side: transpose x per batch, cast to fp16, run SPMD via PJRT on 8 cores,
transpose per-head outputs back.
"""
import sys
sys.path.insert(0, "/opt/trn_rl_repo")
from contextlib import ExitStack

import numpy as np

import concourse.bass as bass
import concourse.mybir as mybir
import concourse.tile as tile
from concourse import bacc
from concourse.bass import ts

F32 = mybir.dt.float32
F16 = mybir.dt.float16
AF = mybir.ActivationFunctionType

B = 8
T = 1024
C = 1024
NH = 16
DH = 64
NP = C // 128   # partition-chunks of C
NT = T // 128   # partition-chunks of T
NQ = T // 512   # moving-dim chunks of T


def _emit(nc, tc, xT_d, wq_d, wk_d, wv_d, bq_d, bk_d, bv_d, out_d):
    with ExitStack() as ctx:
        persist = ctx.enter_context(tc.tile_pool(name="persist", bufs=1))
        expsp = ctx.enter_context(tc.tile_pool(name="expsp", bufs=18))
        rbigp = ctx.enter_context(tc.tile_pool(name="rbigp", bufs=2))
        outp = ctx.enter_context(tc.tile_pool(name="outp", bufs=2))
        # PSUM (8 banks): proj 2 + S 2x2 + O 1x2 = 8
        proj_ps = ctx.enter_context(tc.tile_pool(name="proj_ps", bufs=2, space="PSUM"))
        s_ps = ctx.enter_context(tc.tile_pool(name="s_ps", bufs=2, space="PSUM"))
        o_ps = ctx.enter_context(tc.tile_pool(name="o_ps", bufs=1, space="PSUM"))

        xT_s, wq_s, wk_s, wv_s = [], [], [], []
        for ci in range(NP):
            t = persist.tile([128, T], F16, tag=f"xT{ci}", name=f"xT{ci}")
            nc.sync.dma_start(out=t, in_=xT_d[ts(ci, 128), :])
            xT_s.append(t)
            t = persist.tile([128, C], F16, tag=f"wq{ci}", name=f"wq{ci}")
            nc.sync.dma_start(out=t, in_=wq_d[ts(ci, 128), :])
            wq_s.append(t)
        for ci in range(NP):
            t = persist.tile([128, C], F16, tag=f"wk{ci}", name=f"wk{ci}")
            nc.sync.dma_start(out=t, in_=wk_d[ts(ci, 128), :])
            wk_s.append(t)
        for ci in range(NP):
            t = persist.tile([128, C], F16, tag=f"wv{ci}", name=f"wv{ci}")
            nc.sync.dma_start(out=t, in_=wv_d[ts(ci, 128), :])
            wv_s.append(t)

        bqc, bkc = [], []
        for co in range(NP):
            t = persist.tile([128, 1], F32, tag=f"bq{co}", name=f"bq{co}")
            nc.sync.dma_start(out=t, in_=bq_d[ts(co, 128), :])
            bqc.append(t)
            t = persist.tile([128, 1], F32, tag=f"bk{co}", name=f"bk{co}")
            nc.sync.dma_start(out=t, in_=bk_d[ts(co, 128), :])
            bkc.append(t)
        bv_row = persist.tile([1, C], F32, tag="bv_row", name="bv_row")
        nc.sync.dma_start(out=bv_row, in_=bv_d)
        bvb = []
        for cv in range(2):
            t = persist.tile([128, 512], F32, tag=f"bvb{cv}", name=f"bvb{cv}")
            nc.gpsimd.partition_broadcast(t, bv_row[:, ts(cv, 512)])
            bvb.append(t)

        QT_s = [persist.tile([128, T], F16, tag=f"QT{i}", name=f"QT{i}")
                for i in range(NP)]
        KT_s = [persist.tile([128, T], F16, tag=f"KT{i}", name=f"KT{i}")
                for i in range(NP)]
        # V interleaved per head: [V_h (64 cols) | ones (64 cols)] x 16 heads.
        # One accumulation group then yields O' on psum partitions 0:64 and
        # softmax denominators broadcast on 64:128, with a contiguous lhsT.
        V_s = [persist.tile([128, 2 * C], F16, tag=f"V{i}", name=f"V{i}")
               for i in range(NT)]
        for i in range(NT):
            ones_ap = bass.AP(
                tensor=V_s[i].tensor, offset=V_s[i].offset,
                ap=[V_s[i].ap[0], [2 * DH, NH], [1, DH]])
            nc.vector.memset(ones_ap, 1.0)

        def proj(w_s, bias, co, dst):
            for tq in range(NQ):
                ps = proj_ps.tile([128, 512], F32, tag="proj", name="proj")
                for ci in range(NP):
                    nc.tensor.matmul(
                        ps, w_s[ci][:, ts(co, 128)], xT_s[ci][:, ts(tq, 512)],
                        start=(ci == 0), stop=(ci == NP - 1))
                nc.vector.tensor_scalar_add(dst[:, ts(tq, 512)], ps, bias)

        def proj_v(tt, cv):
            ps = proj_ps.tile([128, 512], F32, tag="proj", name="proj")
            for ci in range(NP):
                nc.tensor.matmul(
                    ps, xT_s[ci][:, ts(tt, 128)], wv_s[ci][:, ts(cv, 512)],
                    start=(ci == 0), stop=(ci == NP - 1))
            dst = bass.AP(
                tensor=V_s[tt].tensor,
                offset=V_s[tt].offset + cv * 1024 + DH,
                ap=[V_s[tt].ap[0], [2 * DH, 8], [1, DH]])
            nc.vector.tensor_add(
                dst, ps.rearrange("p (a b) -> p a b", b=DH),
                bvb[cv].rearrange("p (a b) -> p a b", b=DH))

        def qk_pair(p):
            exps = [[None] * NT for _ in range(2)]
            for tk in range(NT):
                for hh in range(2):
                    sp = s_ps.tile([128, T], F32, tag="S", name="S")
                    for tq in range(NQ):
                        nc.tensor.matmul(
                            sp[:, ts(tq, 512)],
                            KT_s[p][ts(hh, 64), ts(tk, 128)],
                            QT_s[p][ts(hh, 64), ts(tq, 512)],
                            start=True, stop=True,
                            tile_position=(hh * 64, 0))
                    e = expsp.tile([128, T], F16, tag="expS", name="expS")
                    nc.scalar.activation(out=e, in_=sp, func=AF.Exp, scale=0.125)
                    exps[hh][tk] = e
            return exps

        def attend(h, exps_h):
            op = o_ps.tile([128, T], F32, tag="O", name="O")
            for tq in range(NQ):
                for tk in range(NT):
                    nc.tensor.matmul(
                        op[:, ts(tq, 512)], V_s[tk][:, ts(h, 2 * DH)],
                        exps_h[tk][:, ts(tq, 512)],
                        start=(tk == 0), stop=(tk == NT - 1))
            rbig = rbigp.tile([64, T], F32, tag="rbig", name="rbig")
            nc.vector.reciprocal_approx_fast(out=rbig, in_=op[0:64, :])
            stage = outp.tile([64, T], F32, tag="stage", name="stage")
            nc.vector.tensor_mul(stage, op[64:128, :], rbig)
            nc.sync.dma_start(out=out_d[h], in_=stage)

        proj(wq_s, bqc[0], 0, QT_s[0])
        proj(wk_s, bkc[0], 0, KT_s[0])
        prev = (0, qk_pair(0))
        proj(wq_s, bqc[1], 1, QT_s[1])
        proj(wk_s, bkc[1], 1, KT_s[1])
        for tt in range(NT):
            proj_v(tt, 0)
        for p in range(1, NP):
            cur = (p, qk_pair(p))
            if p + 1 < NP:
                proj(wq_s, bqc[p + 1], p + 1, QT_s[p + 1])
                proj(wk_s, bkc[p + 1], p + 1, KT_s[p + 1])
            pp, pe = prev
            attend(2 * pp, pe[0])
            attend(2 * pp + 1, pe[1])
            if p == 3:
                for tt in range(NT):
                    proj_v(tt, 1)
            prev = cur
        pp, pe = prev
        attend(2 * pp, pe[0])
        attend(2 * pp + 1, pe[1])


def _build(repeat: int = 1):
    nc = bacc.Bacc("TRN2", target_bir_lowering=False, debug=False,
                   enable_asserts=False, num_devices=8)
    xT_d = nc.dram_tensor("xT", [C, T], F16, kind="ExternalInput").ap()
    wq_d = nc.dram_tensor("wq", [C, C], F16, kind="ExternalInput").ap()
    wk_d = nc.dram_tensor("wk", [C, C], F16, kind="ExternalInput").ap()
    wv_d = nc.dram_tensor("wv", [C, C], F16, kind="ExternalInput").ap()
    bq_d = nc.dram_tensor("bq", [C, 1], F32, kind="ExternalInput").ap()
    bk_d = nc.dram_tensor("bk", [C, 1], F32, kind="ExternalInput").ap()
    bv_d = nc.dram_tensor("bv", [1, C], F32, kind="ExternalInput").ap()
    out_d = nc.dram_tensor("out", [NH, DH, T], F32, kind="ExternalOutput").ap()
    with tile.TileContext(nc) as tc:
        for _ in range(repeat):
            _emit(nc, tc, xT_d, wq_d, wk_d, wv_d, bq_d, bk_d, bv_d, out_d)
    nc.compile()
    return nc


# ---------------------------------------------------------------- PJRT runner
class _SpmdRunner:
    def __init__(self, nc, n_cores=8):
        import jax
        from jax.sharding import Mesh, PartitionSpec
        from jax.experimental.shard_map import shard_map
        from concourse.bass2jax import (
            _bass_exec_p, install_neuronx_cc_hook, partition_id_tensor)

        install_neuronx_cc_hook()
        self.n_cores = n_cores
        partition_name = (nc.partition_id_tensor.name
                          if nc.partition_id_tensor else None)
        in_names, out_names, out_avals, zero_outs = [], [], [], []
        for alloc in nc.m.functions[0].allocations:
            if not isinstance(alloc, mybir.MemoryLocationSet):
                continue
            name = alloc.memorylocations[0].name
            if alloc.kind == "ExternalInput":
                if name != partition_name:
                    in_names.append(name)
            elif alloc.kind == "ExternalOutput":
                shape = tuple(alloc.tensor_shape)
                dtype = mybir.dt.np(alloc.dtype)
                out_avals.append(jax.core.ShapedArray(shape, dtype))
                out_names.append(name)
                zero_outs.append(np.zeros(shape, dtype))
        self.in_names, self.out_names = in_names, out_names
        self.out_avals, self.zero_outs = out_avals, zero_outs
        n_params, n_outs = len(in_names), len(out_avals)
        all_in_names = list(in_names) + list(out_names)
        if partition_name is not None:
            all_in_names.append(partition_name)

        def _body(*args):
            operands = list(args)
            if partition_name is not None:
                operands.append(partition_id_tensor())
            outs = _bass_exec_p.bind(
                *operands,
                out_avals=tuple(out_avals),
                in_names=tuple(all_in_names),
                out_names=tuple(out_names),
                lowering_input_output_aliases=(),
                sim_require_finite=True,
                sim_require_nnan=True,
                nc=nc,
            )
            return tuple(outs)

        devices = jax.devices()[:n_cores]
        assert len(devices) == n_cores, (
            f"need {n_cores} cores, have {len(jax.devices())}")
        mesh = Mesh(np.asarray(devices), ("core",))
        in_specs = (PartitionSpec("core"),) * (n_params + n_outs)
        out_specs = (PartitionSpec("core"),) * n_outs
        self._fn = jax.jit(
            shard_map(_body, mesh=mesh, in_specs=in_specs,
                      out_specs=out_specs, check_rep=False),
            keep_unused=True)
        self._jax = jax

    def run(self, in_maps):
        n = self.n_cores
        concat_in = [
            np.concatenate([np.asarray(in_maps[c][k]) for c in range(n)], axis=0)
            for k in self.in_names
        ]
        concat_zero = [
            np.zeros((n * z.shape[0], *z.shape[1:]), z.dtype)
            for z in self.zero_outs
        ]
        outs = self._fn(*concat_in, *concat_zero)
        self._jax.block_until_ready(outs)
        return [
            {k: np.asarray(outs[i]).reshape(n, *self.out_avals[i].shape)[c]
             for i, k in enumerate(self.out_names)}
            for c in range(n)
        ]


_CACHE = {}


def kernel(x, Wq, bq, Wk, bk, Wv, bv):
    x = np.asarray(x)
    if "runner" not in _CACHE:
        _CACHE["runner"] = _SpmdRunner(_build(repeat=1), B)
    runner = _CACHE["runner"]

    f16 = np.float16
    wq16, wk16, wv16 = (np.asarray(w).astype(f16) for w in (Wq, Wk, Wv))
    bq2 = np.asarray(bq).reshape(C, 1).astype(np.float32)
    bk2 = np.asarray(bk).reshape(C, 1).astype(np.float32)
    bv2 = np.asarray(bv).reshape(1, C).astype(np.float32)
    in_maps = [{
        "xT": np.ascontiguousarray(x[b].T).astype(f16),
        "wq": wq16, "wk": wk16, "wv": wv16,
        "bq": bq2, "bk": bk2, "bv": bv2,
    } for b in range(B)]

    res = runner.run(in_maps)
    out = np.stack([
        res[b]["out"].transpose(2, 0, 1).reshape(T, C) for b in range(B)
    ]).astype(np.float32)
    return out


# revision 4
# speedup vs baseline: 307.0255x; 307.0255x over previous
"""nn_AttentionHeads_52269751992484 — Trainium2 Bass kernel (8 NeuronCores).

Multi-head attention (non-causal, 16 heads of 64), B=8, T=1024, C=1024.
Sharding: data-parallel over batch — one batch element per NeuronCore, no
collectives. Inside each core (all matmuls fp16 with fp32 PSUM accumulate):

  Q^T = Wq^T x^T (+bq)   K^T likewise       [C, T], head-pair partition tiles
  V   = x Wv (+bv) with a 64-wide all-ones block appended ([128, C+64] tiles)
  per head pair: S^T[tk] = K_h^T-chunk^T @ Q_h^T  (two heads row-packed in the
    PE array via tile_position), exp(S/8) on the scalar engine PSUM->SBUF fp16
  per head: one accumulation group [V_h | ones]^T @ expS gives O' on psum
    partitions 0:64 and the softmax denominators broadcast on 64:128;
    reciprocal_approx_fast + tensor_mul normalize; DMA out per-head O^T.

Host side: transpose x per batch, cast to fp16, run SPMD via PJRT on 8 cores,
transpose per-head outputs back.
"""
import sys
sys.path.insert(0, "/opt/trn_rl_repo")
from contextlib import ExitStack

import numpy as np

import concourse.bass as bass
import concourse.mybir as mybir
import concourse.tile as tile
from concourse import bacc
from concourse.bass import ts

F32 = mybir.dt.float32
F16 = mybir.dt.float16
AF = mybir.ActivationFunctionType

B = 8
T = 1024
C = 1024
NH = 16
DH = 64
NP = C // 128   # partition-chunks of C
NT = T // 128   # partition-chunks of T
NQ = T // 512   # moving-dim chunks of T


def _emit(nc, tc, xT_d, wq_d, wk_d, wv_d, bq_d, bk_d, bv_d, out_d):
    with ExitStack() as ctx:
        persist = ctx.enter_context(tc.tile_pool(name="persist", bufs=1))
        expsp = ctx.enter_context(tc.tile_pool(name="expsp", bufs=18))
        rbigp = ctx.enter_context(tc.tile_pool(name="rbigp", bufs=3))
        outp = ctx.enter_context(tc.tile_pool(name="outp", bufs=3))
        # PSUM (8 banks): proj 2 + S 2x2 + O 2x1 = 8
        proj_ps = ctx.enter_context(tc.tile_pool(name="proj_ps", bufs=2, space="PSUM"))
        s_ps = ctx.enter_context(tc.tile_pool(name="s_ps", bufs=2, space="PSUM"))
        o_ps = ctx.enter_context(tc.tile_pool(name="o_ps", bufs=2, space="PSUM"))

        xT_s, wq_s, wk_s, wv_s = [], [], [], []
        for ci in range(NP):
            t = persist.tile([128, T], F16, tag=f"xT{ci}", name=f"xT{ci}")
            nc.sync.dma_start(out=t, in_=xT_d[ts(ci, 128), :])
            xT_s.append(t)
            t = persist.tile([128, C], F16, tag=f"wq{ci}", name=f"wq{ci}")
            nc.sync.dma_start(out=t, in_=wq_d[ts(ci, 128), :])
            wq_s.append(t)
        for ci in range(NP):
            t = persist.tile([128, C], F16, tag=f"wk{ci}", name=f"wk{ci}")
            nc.sync.dma_start(out=t, in_=wk_d[ts(ci, 128), :])
            wk_s.append(t)
        for ci in range(NP):
            t = persist.tile([128, C], F16, tag=f"wv{ci}", name=f"wv{ci}")
            nc.sync.dma_start(out=t, in_=wv_d[ts(ci, 128), :])
            wv_s.append(t)

        bqc, bkc = [], []
        for co in range(NP):
            t = persist.tile([128, 1], F32, tag=f"bq{co}", name=f"bq{co}")
            nc.sync.dma_start(out=t, in_=bq_d[ts(co, 128), :])
            bqc.append(t)
            t = persist.tile([128, 1], F32, tag=f"bk{co}", name=f"bk{co}")
            nc.sync.dma_start(out=t, in_=bk_d[ts(co, 128), :])
            bkc.append(t)
        bv_row = persist.tile([1, C], F32, tag="bv_row", name="bv_row")
        nc.sync.dma_start(out=bv_row, in_=bv_d)
        bvb = []
        for cv in range(2):
            t = persist.tile([128, 512], F32, tag=f"bvb{cv}", name=f"bvb{cv}")
            nc.gpsimd.partition_broadcast(t, bv_row[:, ts(cv, 512)])
            bvb.append(t)

        QT_s = [persist.tile([128, T], F16, tag=f"QT{i}", name=f"QT{i}")
                for i in range(NP)]
        KT_s = [persist.tile([128, T], F16, tag=f"KT{i}", name=f"KT{i}")
                for i in range(NP)]
        # V interleaved per head: [V_h (64 cols) | ones (64 cols)] x 16 heads.
        # One accumulation group then yields O' on psum partitions 0:64 and
        # softmax denominators broadcast on 64:128, with a contiguous lhsT.
        V_s = [persist.tile([128, 2 * C], F16, tag=f"V{i}", name=f"V{i}")
               for i in range(NT)]
        for i in range(NT):
            ones_ap = bass.AP(
                tensor=V_s[i].tensor, offset=V_s[i].offset,
                ap=[V_s[i].ap[0], [2 * DH, NH], [1, DH]])
            nc.vector.memset(ones_ap, 1.0)

        def proj(w_s, bias, co, dst):
            for tq in range(NQ):
                ps = proj_ps.tile([128, 512], F32, tag="proj", name="proj")
                for ci in range(NP):
                    nc.tensor.matmul(
                        ps, w_s[ci][:, ts(co, 128)], xT_s[ci][:, ts(tq, 512)],
                        start=(ci == 0), stop=(ci == NP - 1))
                nc.vector.tensor_scalar_add(dst[:, ts(tq, 512)], ps, bias)

        def proj_v(tt, cv):
            ps = proj_ps.tile([128, 512], F32, tag="proj", name="proj")
            for ci in range(NP):
                nc.tensor.matmul(
                    ps, xT_s[ci][:, ts(tt, 128)], wv_s[ci][:, ts(cv, 512)],
                    start=(ci == 0), stop=(ci == NP - 1))
            dst = bass.AP(
                tensor=V_s[tt].tensor,
                offset=V_s[tt].offset + cv * 1024 + DH,
                ap=[V_s[tt].ap[0], [2 * DH, 8], [1, DH]])
            nc.vector.tensor_add(
                dst, ps.rearrange("p (a b) -> p a b", b=DH),
                bvb[cv].rearrange("p (a b) -> p a b", b=DH))

        def qk_pair(p):
            exps = [[None] * NT for _ in range(2)]
            for tk in range(NT):
                for hh in range(2):
                    sp = s_ps.tile([128, T], F32, tag="S", name="S")
                    for tq in range(NQ):
                        nc.tensor.matmul(
                            sp[:, ts(tq, 512)],
                            KT_s[p][ts(hh, 64), ts(tk, 128)],
                            QT_s[p][ts(hh, 64), ts(tq, 512)],
                            start=True, stop=True,
                            tile_position=(hh * 64, 0))
                    e = expsp.tile([128, T], F16, tag="expS", name="expS")
                    nc.scalar.activation(out=e, in_=sp, func=AF.Exp, scale=0.125)
                    exps[hh][tk] = e
            return exps

        def attend(h, exps_h):
            # per 512-col half: 1-bank psum, double-buffered so the next AV
            # group never waits on this one's normalization chain
            for tq in range(NQ):
                op = o_ps.tile([128, 512], F32, tag="O", name="O")
                for tk in range(NT):
                    nc.tensor.matmul(
                        op, V_s[tk][:, ts(h, 2 * DH)],
                        exps_h[tk][:, ts(tq, 512)],
                        start=(tk == 0), stop=(tk == NT - 1))
                rbig = rbigp.tile([64, 512], F32, tag="rbig", name="rbig")
                nc.vector.reciprocal_approx_fast(out=rbig, in_=op[0:64, :])
                stage = outp.tile([64, 512], F32, tag="stage", name="stage")
                nc.vector.tensor_mul(stage, op[64:128, :], rbig)
                nc.sync.dma_start(out=out_d[h][:, ts(tq, 512)], in_=stage)

        proj(wq_s, bqc[0], 0, QT_s[0])
        proj(wk_s, bkc[0], 0, KT_s[0])
        prev = (0, qk_pair(0))
        proj(wq_s, bqc[1], 1, QT_s[1])
        proj(wk_s, bkc[1], 1, KT_s[1])
        for tt in range(NT):
            proj_v(tt, 0)
        for p in range(1, NP):
            cur = (p, qk_pair(p))
            if p + 1 < NP:
                proj(wq_s, bqc[p + 1], p + 1, QT_s[p + 1])
                proj(wk_s, bkc[p + 1], p + 1, KT_s[p + 1])
            pp, pe = prev
            attend(2 * pp, pe[0])
            attend(2 * pp + 1, pe[1])
            if p == 3:
                for tt in range(NT):
                    proj_v(tt, 1)
            prev = cur
        pp, pe = prev
        attend(2 * pp, pe[0])
        attend(2 * pp + 1, pe[1])


def _build(repeat: int = 1):
    nc = bacc.Bacc("TRN2", target_bir_lowering=False, debug=False,
                   enable_asserts=False, num_devices=8)
    xT_d = nc.dram_tensor("xT", [C, T], F16, kind="ExternalInput").ap()
    wq_d = nc.dram_tensor("wq", [C, C], F16, kind="ExternalInput").ap()
    wk_d = nc.dram_tensor("wk", [C, C], F16, kind="ExternalInput").ap()
    wv_d = nc.dram_tensor("wv", [C, C], F16, kind="ExternalInput").ap()
    bq_d = nc.dram_tensor("bq", [C, 1], F32, kind="ExternalInput").ap()
    bk_d = nc.dram_tensor("bk", [C, 1], F32, kind="ExternalInput").ap()
    bv_d = nc.dram_tensor("bv", [1, C], F32, kind="ExternalInput").ap()
    out_d = nc.dram_tensor("out", [NH, DH, T], F32, kind="ExternalOutput").ap()
    with tile.TileContext(nc) as tc:
        for _ in range(repeat):
            _emit(nc, tc, xT_d, wq_d, wk_d, wv_d, bq_d, bk_d, bv_d, out_d)
    nc.compile()
    return nc


# ---------------------------------------------------------------- PJRT runner
class _SpmdRunner:
    def __init__(self, nc, n_cores=8):
        import jax
        from jax.sharding import Mesh, PartitionSpec
        from jax.experimental.shard_map import shard_map
        from concourse.bass2jax import (
            _bass_exec_p, install_neuronx_cc_hook, partition_id_tensor)

        install_neuronx_cc_hook()
        self.n_cores = n_cores
        partition_name = (nc.partition_id_tensor.name
                          if nc.partition_id_tensor else None)
        in_names, out_names, out_avals, zero_outs = [], [], [], []
        for alloc in nc.m.functions[0].allocations:
            if not isinstance(alloc, mybir.MemoryLocationSet):
                continue
            name = alloc.memorylocations[0].name
            if alloc.kind == "ExternalInput":
                if name != partition_name:
                    in_names.append(name)
            elif alloc.kind == "ExternalOutput":
                shape = tuple(alloc.tensor_shape)
                dtype = mybir.dt.np(alloc.dtype)
                out_avals.append(jax.core.ShapedArray(shape, dtype))
                out_names.append(name)
                zero_outs.append(np.zeros(shape, dtype))
        self.in_names, self.out_names = in_names, out_names
        self.out_avals, self.zero_outs = out_avals, zero_outs
        n_params, n_outs = len(in_names), len(out_avals)
        all_in_names = list(in_names) + list(out_names)
        if partition_name is not None:
            all_in_names.append(partition_name)

        def _body(*args):
            operands = list(args)
            if partition_name is not None:
                operands.append(partition_id_tensor())
            outs = _bass_exec_p.bind(
                *operands,
                out_avals=tuple(out_avals),
                in_names=tuple(all_in_names),
                out_names=tuple(out_names),
                lowering_input_output_aliases=(),
                sim_require_finite=True,
                sim_require_nnan=True,
                nc=nc,
            )
            return tuple(outs)

        devices = jax.devices()[:n_cores]
        assert len(devices) == n_cores, (
            f"need {n_cores} cores, have {len(jax.devices())}")
        mesh = Mesh(np.asarray(devices), ("core",))
        in_specs = (PartitionSpec("core"),) * (n_params + n_outs)
        out_specs = (PartitionSpec("core"),) * n_outs
        self._fn = jax.jit(
            shard_map(_body, mesh=mesh, in_specs=in_specs,
                      out_specs=out_specs, check_rep=False),
            keep_unused=True)
        self._jax = jax

    def run(self, in_maps):
        n = self.n_cores
        concat_in = [
            np.concatenate([np.asarray(in_maps[c][k]) for c in range(n)], axis=0)
            for k in self.in_names
        ]
        concat_zero = [
            np.zeros((n * z.shape[0], *z.shape[1:]), z.dtype)
            for z in self.zero_outs
        ]
        outs = self._fn(*concat_in, *concat_zero)
        self._jax.block_until_ready(outs)
        return [
            {k: np.asarray(outs[i]).reshape(n, *self.out_avals[i].shape)[c]
             for i, k in enumerate(self.out_names)}
            for c in range(n)
        ]


_CACHE = {}


def kernel(x, Wq, bq, Wk, bk, Wv, bv):
    x = np.asarray(x)
    if "runner" not in _CACHE:
        _CACHE["runner"] = _SpmdRunner(_build(repeat=1), B)
    runner = _CACHE["runner"]

    f16 = np.float16
    wq16, wk16, wv16 = (np.asarray(w).astype(f16) for w in (Wq, Wk, Wv))
    bq2 = np.asarray(bq).reshape(C, 1).astype(np.float32)
    bk2 = np.asarray(bk).reshape(C, 1).astype(np.float32)
    bv2 = np.asarray(bv).reshape(1, C).astype(np.float32)
    in_maps = [{
        "xT": np.ascontiguousarray(x[b].T).astype(f16),
        "wq": wq16, "wk": wk16, "wv": wv16,
        "bq": bq2, "bk": bk2, "bv": bv2,
    } for b in range(B)]

    res = runner.run(in_maps)
    out = np.stack([
        res[b]["out"].transpose(2, 0, 1).reshape(T, C) for b in range(B)
    ]).astype(np.float32)
    return out
